# revision 1
# baseline (speedup 1.0000x reference)
"""MoE runtime-experts kernel for 8 Trainium2 NeuronCores.

Expert-parallel: core e holds expert e's weights. Host routes tokens by
expert id (argsort), pads each expert batch to a common capacity C, and
each core computes y = gelu(x @ W1 + b1) @ W2 + b2 for its batch as
dense matmuls in a transposed layout:

    L1: hT[hid, tok]  = W1[in, hid].T-contract  xT[in, tok]
    L2: yT[out, tok]  = W2[hid, out].T-contract hT[hid, tok]

Weights stay stationary on the PE (lhsT), tokens are the moving dim, so
activations flow through both layers without any on-device transpose.
Inputs/weights are cast to bf16 on host (PSUM accumulates fp32).
"""

import numpy as np
import ml_dtypes

import concourse.bass as bass
import concourse.mybir as mybir
import concourse.tile as tile
from concourse import bacc
from concourse.bass_utils import run_bass_kernel_spmd

P = 128
N_CORES = 8
BF16 = mybir.dt.bfloat16
F32 = mybir.dt.float32

_nc_cache = {}


def _token_tile_widths(C):
    widths = [512] * (C // 512)
    if C % 512:
        widths.append(C % 512)
    return widths


def _build_kernel(C, IN, HID, OUT):
    K1, M1 = IN // P, HID // P
    K2, M2 = HID // P, OUT // P
    widths = _token_tile_widths(C)

    nc = bacc.Bacc("TRN2", target_bir_lowering=False, debug=False,
                   num_devices=N_CORES)
    xT = nc.dram_tensor("xT", [IN, C], BF16, kind="ExternalInput")
    w1 = nc.dram_tensor("w1", [IN, HID], BF16, kind="ExternalInput")
    w2 = nc.dram_tensor("w2", [HID, OUT], BF16, kind="ExternalInput")
    b1 = nc.dram_tensor("b1", [HID], F32, kind="ExternalInput")
    b2 = nc.dram_tensor("b2", [OUT], F32, kind="ExternalInput")
    yT = nc.dram_tensor("yT", [OUT, C], F32, kind="ExternalOutput")

    with tile.TileContext(nc) as tc:
        with (
            tc.tile_pool(name="weights", bufs=1) as wpool,
            tc.tile_pool(name="xbuf", bufs=2) as xpool,
            tc.tile_pool(name="hbuf", bufs=1) as hpool,
            tc.tile_pool(name="obuf", bufs=4) as opool,
            tc.tile_pool(name="psum", bufs=4, space="PSUM") as pspool,
        ):
            w1_sb = wpool.tile([P, K1, HID], BF16)
            w2_sb = wpool.tile([P, K2, OUT], BF16)
            b1_sb = wpool.tile([P, M1], F32)
            b2_sb = wpool.tile([P, M2], F32)

            nc.sync.dma_start(b1_sb[:], b1.ap().rearrange("(m p) -> p m", p=P))
            nc.sync.dma_start(b2_sb[:], b2.ap().rearrange("(m p) -> p m", p=P))
            w1r = w1.ap().rearrange("(k p) m -> p k m", p=P)
            for k in range(K1):
                nc.sync.dma_start(w1_sb[:, k], w1r[:, k])
            w2r = w2.ap().rearrange("(k p) m -> p k m", p=P)
            for k in range(K2):
                nc.sync.dma_start(w2_sb[:, k], w2r[:, k])
            xTr = xT.ap().rearrange("(k p) c -> p k c", p=P)

            n0 = 0
            for NW in widths:
                x_sb = xpool.tile([P, K1, 512], BF16, tag="x")
                nc.sync.dma_start(x_sb[:, :, :NW], xTr[:, :, n0:n0 + NW])

                h_sb = hpool.tile([P, K2, 512], BF16, tag="h")
                for m in range(M1):
                    ps = pspool.tile([P, 512], F32, tag="ps")
                    for k in range(K1):
                        nc.tensor.matmul(
                            ps[:, :NW],
                            w1_sb[:, k, bass.ts(m, P)],
                            x_sb[:, k, :NW],
                            start=(k == 0),
                            stop=(k == K1 - 1),
                        )
                    nc.scalar.activation(
                        h_sb[:, m, :NW],
                        ps[:, :NW],
                        mybir.ActivationFunctionType.Gelu,
                        bias=b1_sb[:, m:m + 1],
                    )
                for m in range(M2):
                    ps = pspool.tile([P, 512], F32, tag="ps")
                    for k in range(K2):
                        nc.tensor.matmul(
                            ps[:, :NW],
                            w2_sb[:, k, bass.ts(m, P)],
                            h_sb[:, k, :NW],
                            start=(k == 0),
                            stop=(k == K2 - 1),
                        )
                    o_sb = opool.tile([P, 512], F32, tag="o")
                    nc.vector.tensor_tensor(
                        o_sb[:, :NW],
                        ps[:, :NW],
                        b2_sb[:, m:m + 1].to_broadcast((P, NW)),
                        mybir.AluOpType.add,
                    )
                    nc.sync.dma_start(yT.ap()[bass.ts(m, P), n0:n0 + NW],
                                      o_sb[:, :NW])
                n0 += NW
    nc.compile()
    return nc


def _get_kernel(C, IN, HID, OUT):
    key = (C, IN, HID, OUT)
    if key not in _nc_cache:
        _nc_cache[key] = _build_kernel(C, IN, HID, OUT)
    return _nc_cache[key]


def kernel(**inputs):
    x = np.ascontiguousarray(np.asarray(inputs["x"], dtype=np.float32))
    idx = np.asarray(inputs["indices_s"]).astype(np.int64)
    w1 = np.asarray(inputs["weight1"], dtype=np.float32)
    w2 = np.asarray(inputs["weight2"], dtype=np.float32)
    b1 = np.asarray(inputs["bias1"], dtype=np.float32)
    b2 = np.asarray(inputs["bias2"], dtype=np.float32)

    T = x.shape[0]
    E, IN, HID = w1.shape
    OUT = w2.shape[2]
    assert E == N_CORES
    bf = ml_dtypes.bfloat16

    order = np.argsort(idx, kind="stable")
    counts = np.bincount(idx, minlength=E)
    starts = np.zeros(E + 1, dtype=np.int64)
    starts[1:] = np.cumsum(counts)
    C = max(-(-int(counts.max()) // P) * P, P)

    nc = _get_kernel(C, IN, HID, OUT)

    xbf = x.astype(bf)
    in_maps = []
    for e in range(E):
        toks = order[starts[e]:starts[e + 1]]
        xTe = np.zeros((IN, C), dtype=bf)
        if len(toks):
            xTe[:, :len(toks)] = xbf[toks].T
        in_maps.append({
            "xT": xTe,
            "w1": np.ascontiguousarray(w1[e]).astype(bf),
            "w2": np.ascontiguousarray(w2[e]).astype(bf),
            "b1": np.ascontiguousarray(b1[e]),
            "b2": np.ascontiguousarray(b2[e]),
        })

    res = run_bass_kernel_spmd(nc, in_maps, core_ids=list(range(N_CORES)),
                               trace=False)

    out = np.empty((T, OUT), dtype=np.float32)
    for e in range(E):
        toks = order[starts[e]:starts[e + 1]]
        if len(toks):
            out[toks] = res.results[e]["yT"][:, :len(toks)].T
    return out[:, None, :]


# revision 23
# speedup vs baseline: 15347.4302x; 15347.4302x over previous
"""MoE runtime-experts kernel for 8 Trainium2 NeuronCores.

Expert-parallel: core e holds expert e's weights. Host routes tokens by
expert id (argsort), pads each expert batch to a common capacity C, and
each core computes y = gelu(x @ W1 + b1) @ W2 + b2 for its batch as
dense matmuls in a transposed layout:

    L1: hT[hid, tok]  = W1[in, hid].T-contract  xT[in, tok]
    L2: yT[out, tok]  = W2[hid, out].T-contract hT[hid, tok]

Weights stay stationary on the PE (lhsT), tokens are the moving dim, so
activations flow through both layers without any on-device transpose.
Inputs/weights are cast to bf16 on host (PSUM accumulates fp32).
"""

import numpy as np
import ml_dtypes

import concourse.bass as bass
import concourse.mybir as mybir
import concourse.tile as tile
from concourse import bacc
from concourse.bass_utils import run_bass_kernel_spmd

P = 128
N_CORES = 8
BF16 = mybir.dt.bfloat16
F32 = mybir.dt.float32

_nc_cache = {}


def _token_tile_widths(C):
    """Split C (multiple of 128) into near-equal 128-multiple chunks <=512.
    Equal chunks keep every matmul's moving dim >=256 (for C>=512), so the
    per-matmul LDWEIGHTS (107ns) stays hidden under the MM stream."""
    nch = -(-C // 512)
    base = (C // nch) // P * P
    widths = [base] * nch
    rem = (C - base * nch) // P
    for i in range(rem):
        widths[i] += P
    assert sum(widths) == C
    return widths


def _build_kernel(C, IN, HID, OUT, skip_in_dma=False, psum_bufs=8, repeat=1,
                  tile_w=128, PIPE=3):
    K1, M1 = IN // P, HID // P
    K2, M2 = HID // P, OUT // P
    if tile_w is not None:
        assert C % tile_w == 0
        widths = [tile_w] * (C // tile_w)
    else:
        widths = _token_tile_widths(C)

    nc = bacc.Bacc("TRN2", target_bir_lowering=False, debug=False,
                   num_devices=N_CORES)
    xT = nc.dram_tensor("xT", [IN, C], BF16, kind="ExternalInput")
    w1 = nc.dram_tensor("w1", [IN, HID], BF16, kind="ExternalInput")
    w2 = nc.dram_tensor("w2", [HID, OUT], BF16, kind="ExternalInput")
    b1 = nc.dram_tensor("b1", [HID], F32, kind="ExternalInput")
    b2 = nc.dram_tensor("b2", [OUT], F32, kind="ExternalInput")
    yT = nc.dram_tensor("yT", [OUT, C], F32, kind="ExternalOutput")

    with tile.TileContext(nc) as tc:
        with (
            tc.tile_pool(name="weights", bufs=1) as wpool,
            tc.tile_pool(name="xbuf", bufs=PIPE + 1) as xpool,
            tc.tile_pool(name="hbuf", bufs=PIPE + 1) as hpool,
            tc.tile_pool(name="obuf", bufs=2) as opool,
            tc.tile_pool(name="psum", bufs=psum_bufs, space="PSUM") as pspool,
        ):
            w1_sb = wpool.tile([P, K1, HID], BF16)
            w2_sb = wpool.tile([P, K2, OUT], BF16)
            b1_sb = wpool.tile([P, M1], F32)
            b2_sb = wpool.tile([P, M2], F32)

            xTr = xT.ap().rearrange("(k p) c -> p k c", p=P)
            w1r = w1.ap().rearrange("(k p) m -> p k m", p=P)
            w2r = w2.ap().rearrange("(k p) m -> p k m", p=P)

            # First token tile's x up front so PE can start ASAP.
            MAXW_ = max(widths)
            x_tiles = {}
            if repeat == 1:
                x_tiles[0] = xpool.tile([P, K1, MAXW_], BF16, tag="x",
                                        name="x_sb")
            if not skip_in_dma:
                if repeat == 1:
                    nc.sync.dma_start(x_tiles[0][:, :, :widths[0]],
                                      xTr[:, :, 0:widths[0]])
                # Biases are tiny and the first gelu needs b1 early.
                nc.sync.dma_start(b1_sb[:],
                                  b1.ap().rearrange("(m p) -> p m", p=P))
                nc.sync.dma_start(b2_sb[:],
                                  b2.ap().rearrange("(m p) -> p m", p=P))
                # w1 chunked by m-window (all k per DMA): the first L1 chain
                # only needs window 0 (1MB) instead of all of w1 (8MB), and
                # one big DMA per window keeps the per-DMA issue overhead
                # (~0.7us on the SP sequencer) off the critical path.
                MW = 512
                for mw in range(HID // MW):
                    nc.sync.dma_start(
                        w1_sb[:, :, mw * MW:(mw + 1) * MW],
                        w1r[:, :, mw * MW:(mw + 1) * MW])
                # w2 is only needed ~halfway in; 4-k chunks, k-major order.
                KC = 4
                for kc in range(K2 // KC):
                    nc.sync.dma_start(w2_sb[:, kc * KC:(kc + 1) * KC],
                                      w2r[:, kc * KC:(kc + 1) * KC])

            starts_ = [sum(widths[:i]) for i in range(len(widths))]
            MAXW = max(widths)

            def l1_phase(it):
                NW, n0 = widths[it], starts_[it]
                if it in x_tiles:
                    x_sb = x_tiles[it]
                else:
                    x_sb = xpool.tile([P, K1, MAXW], BF16, tag="x",
                                      name="x_sb")
                    if not skip_in_dma:
                        nc.sync.dma_start(x_sb[:, :, :NW],
                                          xTr[:, :, n0:n0 + NW])
                h_sb = hpool.tile([P, K2, MAXW], BF16, tag="h", name="h_sb")
                for m in range(M1):
                    ps = pspool.tile([P, 512], F32, tag="ps", name="ps")
                    for k in range(K1):
                        nc.tensor.matmul(
                            ps[:, :NW],
                            w1_sb[:, k, bass.ts(m, P)],
                            x_sb[:, k, :NW],
                            start=(k == 0),
                            stop=(k == K1 - 1),
                        )
                    nc.scalar.activation(
                        h_sb[:, m, :NW],
                        ps[:, :NW],
                        mybir.ActivationFunctionType.Gelu,
                        bias=b1_sb[:, m:m + 1],
                    )
                return h_sb

            yTr = yT.ap().rearrange("(m p) c -> p m c", p=P)

            def l2_phase(it, h_sb):
                NW, n0 = widths[it], starts_[it]
                o_sb = opool.tile([P, M2, MAXW], F32, tag="o", name="o_sb")
                for m in range(M2):
                    ps = pspool.tile([P, 512], F32, tag="ps", name="ps")
                    for k in range(K2):
                        nc.tensor.matmul(
                            ps[:, :NW],
                            w2_sb[:, k, bass.ts(m, P)],
                            h_sb[:, k, :NW],
                            start=(k == 0),
                            stop=(k == K2 - 1),
                        )
                    nc.vector.tensor_tensor(
                        o_sb[:, m, :NW],
                        ps[:, :NW],
                        b2_sb[:, m:m + 1].to_broadcast((P, NW)),
                        mybir.AluOpType.add,
                    )
                nc.sync.dma_start(yTr[:, :, n0:n0 + NW], o_sb[:, :, :NW])

            def body():
                # Software pipeline: L1 runs PIPE tiles ahead of L2 so the
                # w2 weight DMA tail hides behind L1 compute at startup.
                n_t = len(widths)
                depth = min(PIPE, n_t)
                hs = {}
                for it in range(depth):
                    hs[it] = l1_phase(it)
                for j in range(n_t):
                    if j + depth < n_t:
                        hs[j + depth] = l1_phase(j + depth)
                    l2_phase(j, hs.pop(j))

            if repeat == 1:
                body()
            else:
                with tc.For_i(0, repeat, 1, name="rep"):
                    body()
    nc.compile()
    return nc


def _get_kernel(C, IN, HID, OUT):
    key = (C, IN, HID, OUT)
    if key not in _nc_cache:
        _nc_cache[key] = _build_kernel(C, IN, HID, OUT)
    return _nc_cache[key]


def prepare_in_maps(inputs):
    """Host-side routing: sort tokens by expert, pad to capacity C,
    build per-core input maps. Returns (in_maps, meta)."""
    x = np.ascontiguousarray(np.asarray(inputs["x"], dtype=np.float32))
    idx = np.asarray(inputs["indices_s"]).astype(np.int64)
    w1 = np.asarray(inputs["weight1"], dtype=np.float32)
    w2 = np.asarray(inputs["weight2"], dtype=np.float32)
    b1 = np.asarray(inputs["bias1"], dtype=np.float32)
    b2 = np.asarray(inputs["bias2"], dtype=np.float32)

    T = x.shape[0]
    E, IN, HID = w1.shape
    OUT = w2.shape[2]
    assert E == N_CORES
    bf = ml_dtypes.bfloat16

    order = np.argsort(idx, kind="stable")
    counts = np.bincount(idx, minlength=E)
    starts = np.zeros(E + 1, dtype=np.int64)
    starts[1:] = np.cumsum(counts)
    C = max(-(-int(counts.max()) // P) * P, P)

    xbf = x.astype(bf)
    in_maps = []
    for e in range(E):
        toks = order[starts[e]:starts[e + 1]]
        xTe = np.zeros((IN, C), dtype=bf)
        if len(toks):
            xTe[:, :len(toks)] = xbf[toks].T
        in_maps.append({
            "xT": xTe,
            "w1": np.ascontiguousarray(w1[e]).astype(bf),
            "w2": np.ascontiguousarray(w2[e]).astype(bf),
            "b1": np.ascontiguousarray(b1[e]),
            "b2": np.ascontiguousarray(b2[e]),
        })
    meta = {"key": (C, IN, HID, OUT), "order": order, "starts": starts,
            "T": T, "OUT": OUT}
    return in_maps, meta


def scatter_output(inputs, yT_all, meta):
    """Scatter per-core yT [E, OUT, C] back to [T, 1, OUT] fp32."""
    order, starts = meta["order"], meta["starts"]
    out = np.empty((meta["T"], meta["OUT"]), dtype=np.float32)
    for e in range(N_CORES):
        toks = order[starts[e]:starts[e + 1]]
        if len(toks):
            out[toks] = yT_all[e][:, :len(toks)].T
    return out[:, None, :]


def kernel(**inputs):
    in_maps, meta = prepare_in_maps(inputs)
    nc = _get_kernel(*meta["key"])
    res = run_bass_kernel_spmd(nc, in_maps, core_ids=list(range(N_CORES)),
                               trace=False)
    yT_all = np.stack([res.results[e]["yT"] for e in range(N_CORES)])
    return scatter_output(inputs, yT_all, meta)
